# revision 45
# baseline (speedup 1.0000x reference)
"""DiagMean Trainium2 kernel.

Computes, for each batch b of a [16, 2048, 2048] fp32 tensor, the mean of
each of the 2049 diagonals with offset d in [-1024, 1024] (reference
semantics: each diagonal's LAST element is excluded, count = T-1-|d|),
then centers across diagonals and negates.

Approach (per NeuronCore, data-parallel over batch, 2 batches/core):
  * Host pads each [T, T] matrix into [T, 4096] with 1024 zero columns on
    each side, and zeroes the per-diagonal excluded elements (last element
    of every diagonal = last row / last column band). Pure layout, no math.
  * Device reads "skewed" tiles: tile[p, j] = padded[r0+p, (r0+p) + j]
    (partition stride W+1 elements), so column j holds diagonal d = j-1024
    for every row. Reads are trimmed per row-block to the union of valid
    j-windows; out-of-band positions inside the window are host zeros.
  * Diagonal sums = column sums over all rows: most row-blocks go through
    TensorE (ones[128,1] stationary, fp32 matmul accumulating in PSUM);
    a few interleaved blocks accumulate on VectorE into an SBUF
    accumulator that TensorE folds into PSUM at the end. This keeps the
    PE under the DMA roofline while staying exact fp32.
  * Tail: means_neg = sums * (-1/count); avg_neg = mean(means_neg);
    out = means_neg - avg_neg  ( = avg - means = -(means - avg) ).
"""

import numpy as np

import concourse.bass as bass
import concourse.tile as tile
from concourse import bacc, mybir
from concourse.bass_utils import run_bass_kernel_spmd

B, T = 16, 2048
H = T // 2            # 1024 max |offset|
D = T + 1             # 2049 diagonals
W = T + 2 * H         # 4096 padded row width
NCORES = 8
BPC = B // NCORES     # batches per core
P = 128
NBLK = T // P         # 16 row blocks
FP32 = mybir.dt.float32

# Row blocks whose accumulation runs on VectorE instead of TensorE
# (interleaved so the PE never idles long enough to get HAM-throttled).
# Even blocks carry ~half the total trimmed width; block 15 stays on the
# PE so the accumulator fold can be issued before the last block lands.
DVE_BLOCKS = {0, 2, 4, 6, 8, 10, 12, 14}
# Alternate tile loads between the two HWDGE rings (SP and ACT queues).
SPLIT_DMA_QUEUES = True
# Additionally offload group 2 of PE blocks to the DVE (measured slower:
# the extra per-block DVE op + drain outweighs the PE savings).
STEAL_G2 = False

_cache = {}


def _window(blk):
    """Union of valid j-ranges for rows [r0, r0+127]: j must satisfy
    0 <= r + (j - H) <= T-1 for some row r in the block."""
    r0 = blk * P
    w0 = max(0, H - (r0 + P - 1))
    w1 = min(D, (H + T - 1) - r0 + 1)
    return w0, w1


def _build_nc():
    nc = bacc.Bacc(None, target_bir_lowering=False)
    x = nc.dram_tensor("x", [BPC, T, W], FP32, kind="ExternalInput")
    invc = nc.dram_tensor("invc", [1, D], FP32, kind="ExternalInput")
    out = nc.dram_tensor("out", [BPC, D], FP32, kind="ExternalOutput")

    groups = [(512 * g, min(512 * g + 512, D)) for g in range(5)]

    with tile.TileContext(nc) as tc:
        with (
            tc.tile_pool(name="consts", bufs=1) as consts,
            tc.tile_pool(name="tiles", bufs=12) as tiles,
            tc.tile_pool(name="psum", bufs=1, space="PSUM") as psum,
            tc.tile_pool(name="scratch", bufs=1, space="PSUM") as scratch_pool,
            tc.tile_pool(name="accp", bufs=2) as accp,
            tc.tile_pool(name="tail", bufs=2) as tail,
        ):
            ones = consts.tile([P, 1], FP32)
            nc.vector.memset(ones, 1.0)
            zeros = consts.tile([1, 1], FP32)
            nc.vector.memset(zeros, 0.0)
            invc_t = consts.tile([1, D], FP32)
            nc.sync.dma_start(out=invc_t, in_=invc[:, :])
            scratch = scratch_pool.tile([1, 1], FP32)

            # The fp32 self-loading matmul can carry at most ONE sync wait
            # before bacc has to split waits into event-semaphore chains.
            # Absorber matmuls pull cross-engine ticks into the PE vector
            # clock so real matmuls only ever wait on their tile's DMA.
            def absorb(dep_ap, out_ap=None, start=True):
                nc.tensor.matmul(
                    out=scratch[:, :] if out_ap is None else out_ap,
                    lhsT=ones[0:1, 0:1],
                    rhs=dep_ap,
                    start=start,
                    stop=True,
                    skip_group_check=True,
                )

            absorb(ones[0:1, 0:1])  # waits on the DVE memset

            res_tiles = []
            prev_means = None
            # One PSUM region for both batches: batch 1 keeps accumulating on
            # top of batch 0's sums (no per-batch PSUM reset), and its own
            # sums are recovered as total - batch0 in the tail.
            ps = psum.tile([1, D], FP32)
            means0 = ssum0 = avg0n = None
            for b in range(BPC):
                acc = accp.tile([P, D], FP32)
                nc.gpsimd.memset(acc, 0.0)
                if prev_means is not None:
                    # absorb the DVE read of the previous batch's PSUM so the
                    # next matmuls don't carry a WAR wait on DVE
                    absorb(prev_means[0:1, 0:1])
                else:
                    # Zero every PSUM group with a full-width start=True
                    # matmul against the freshly-memset accumulator. Trimmed
                    # block matmuls can then accumulate at any sub-range:
                    # partial-width start=True would leave a bank's
                    # pending-zero state mixed, which is undefined on HW.
                    for c0, c1 in groups:
                        nc.tensor.matmul(
                            out=ps[:, c0:c1],
                            lhsT=ones[:, :],
                            rhs=acc[:, c0:c1],
                            start=True,
                            stop=False,
                            skip_group_check=True,
                        )
                def do_block(blk, steal_g2=STEAL_G2):
                    w0, w1 = _window(blk)
                    tl = tiles.tile([P, w1 - w0], FP32)
                    off = b * T * W + blk * P * (W + 1) + w0
                    src = bass.AP(
                        tensor=x, offset=off, ap=[[W + 1, P], [1, w1 - w0]]
                    )
                    eng = nc.scalar if (SPLIT_DMA_QUEUES and blk % 2) else nc.sync
                    eng.dma_start(out=tl[:, :], in_=src)
                    if blk in DVE_BLOCKS:
                        nc.vector.tensor_add(
                            out=acc[:, w0:w1], in0=acc[:, w0:w1], in1=tl[:, :]
                        )
                        return
                    # PE block: DVE still takes this block's group-2 slice to
                    # shave the PE (which would otherwise be the bottleneck)
                    # without ever leaving it idle long enough to re-throttle.
                    d0, d1 = max(1024, w0), min(1536, w1)
                    if steal_g2 and d0 < d1:
                        nc.vector.tensor_add(
                            out=acc[:, d0:d1],
                            in0=acc[:, d0:d1],
                            in1=tl[:, d0 - w0 : d1 - w0],
                        )
                    for g, (c0, c1) in enumerate(groups):
                        if g == 2 and steal_g2:
                            continue
                        i0, i1 = max(c0, w0), min(c1, w1)
                        if i0 >= i1:
                            continue
                        nc.tensor.matmul(
                            out=ps[:, i0:i1],
                            lhsT=ones[:, :],
                            rhs=tl[:, i0 - w0 : i1 - w0],
                            start=False,
                            stop=False,
                            skip_group_check=True,
                        )

                for blk in range(NBLK - 1):
                    do_block(blk)
                # fold the DVE accumulator into PSUM before the last block's
                # data has even landed (the acc is complete after block 14)
                for c0, c1 in groups:
                    nc.tensor.matmul(
                        out=ps[:, c0:c1],
                        lhsT=ones[:, :],
                        rhs=acc[:, c0:c1],
                        start=False,
                        stop=False,
                        skip_group_check=True,
                    )
                do_block(NBLK - 1, steal_g2=False)
                # close the accumulation (adds 0; stop is sim-side only)
                nc.tensor.matmul(
                    out=ps[:, 0:1],
                    lhsT=ones[0:1, 0:1],
                    rhs=zeros[:, :],
                    start=False,
                    stop=True,
                    skip_group_check=True,
                )
                means = tail.tile([1, D], FP32)
                ssum = tail.tile([1, 1], FP32)
                # one DVE pass: means_neg = ps * (-1/count), ssum = sum(means_neg)
                nc.vector.scalar_tensor_tensor(
                    out=means,
                    in0=ps[:, :],
                    scalar=1.0,
                    in1=invc_t,
                    op0=mybir.AluOpType.bypass,
                    op1=mybir.AluOpType.mult,
                    accum_out=ssum,
                )
                prev_means = means
                res = tail.tile([1, D], FP32)
                if b == 0:
                    means0, ssum0 = means, ssum
                    avg = tail.tile([1, 1], FP32)
                    nc.scalar.mul(avg, ssum, 1.0 / D)
                    avg0n = tail.tile([1, 1], FP32)
                    nc.scalar.mul(avg0n, ssum, -1.0 / D)
                    nc.vector.tensor_scalar(
                        out=res,
                        in0=means,
                        scalar1=avg,
                        scalar2=None,
                        op0=mybir.AluOpType.subtract,
                    )
                else:
                    # batch1 sums = total - batch0:
                    #   avg1 = ssum_total/D - ssum0/D
                    #   res1 = (means_total - avg1) - means0
                    avg1 = tail.tile([1, 1], FP32)
                    nc.scalar.activation(
                        out=avg1,
                        in_=ssum,
                        func=mybir.ActivationFunctionType.Identity,
                        bias=avg0n,
                        scale=1.0 / D,
                    )
                    nc.vector.scalar_tensor_tensor(
                        out=res,
                        in0=means,
                        scalar=avg1,
                        in1=means0,
                        op0=mybir.AluOpType.subtract,
                        op1=mybir.AluOpType.subtract,
                    )
                res_tiles.append(res)
            for b, res in enumerate(res_tiles):
                nc.sync.dma_start(out=out[b : b + 1, :], in_=res[:, :])
    nc.compile()
    return nc


def _prepare(x):
    """Pad rows to width W with the diagonal band centered, and zero the
    excluded (last) element of every diagonal."""
    x = np.ascontiguousarray(np.asarray(x, dtype=np.float32))
    assert x.shape == (B, T, T)
    xp = np.zeros((B, T, W), np.float32)
    xp[:, :, H : H + T] = x
    # d >= 0: excluded element is (T-1-d, T-1)
    rows = T - 1 - np.arange(0, H + 1)
    xp[:, rows, H + T - 1] = 0.0
    # d < 0: excluded element is (T-1, T-1+d)
    cols = T - 1 + np.arange(-H, 0)
    xp[:, T - 1, H + cols] = 0.0
    return xp


def _run(x, trace=False):
    if "nc" not in _cache:
        _cache["nc"] = _build_nc()
    nc = _cache["nc"]

    xp = _prepare(x)
    counts = (T - 1 - np.abs(np.arange(-H, H + 1))).astype(np.float32)
    invc = (-1.0 / counts).reshape(1, D)

    in_maps = [
        {"x": xp[c * BPC : (c + 1) * BPC], "invc": invc} for c in range(NCORES)
    ]
    r = run_bass_kernel_spmd(nc, in_maps, core_ids=list(range(NCORES)), trace=trace)
    out = np.concatenate([m["out"] for m in r.results], axis=0)
    return out, r.exec_time_ns


def kernel(inputs):
    out, _ = _run(inputs, trace=False)
    return out


# revision 47
# speedup vs baseline: 1.1238x; 1.1238x over previous
"""DiagMean Trainium2 kernel.

Computes, for each batch b of a [16, 2048, 2048] fp32 tensor, the mean of
each of the 2049 diagonals with offset d in [-1024, 1024] (reference
semantics: each diagonal's LAST element is excluded, count = T-1-|d|),
then centers across diagonals and negates.

Approach (per NeuronCore, data-parallel over batch, 2 batches/core):
  * Host pads each [T, T] matrix into [T, 4096] with 1024 zero columns on
    each side, and zeroes the per-diagonal excluded elements (last element
    of every diagonal = last row / last column band). Pure layout, no math.
  * Device reads "skewed" tiles: tile[p, j] = padded[r0+p, (r0+p) + j]
    (partition stride W+1 elements), so column j holds diagonal d = j-1024
    for every row. Reads are trimmed per row-block to the union of valid
    j-windows; out-of-band positions inside the window are host zeros.
  * Diagonal sums = column sums over all rows: most row-blocks go through
    TensorE (ones[128,1] stationary, fp32 matmul accumulating in PSUM);
    a few interleaved blocks accumulate on VectorE into an SBUF
    accumulator that TensorE folds into PSUM at the end. This keeps the
    PE under the DMA roofline while staying exact fp32.
  * Tail: means_neg = sums * (-1/count); avg_neg = mean(means_neg);
    out = means_neg - avg_neg  ( = avg - means = -(means - avg) ).
"""

import numpy as np

import concourse.bass as bass
import concourse.tile as tile
from concourse import bacc, mybir
from concourse.bass_utils import run_bass_kernel_spmd

B, T = 16, 2048
H = T // 2            # 1024 max |offset|
D = T + 1             # 2049 diagonals
W = T + 2 * H         # 4096 padded row width
NCORES = 8
BPC = B // NCORES     # batches per core
P = 128
NBLK = T // P         # 16 row blocks
FP32 = mybir.dt.float32

# Row blocks whose accumulation runs on VectorE instead of TensorE
# (interleaved so the PE never idles long enough to get HAM-throttled).
# Even blocks carry ~half the total trimmed width; block 15 stays on the
# PE so the accumulator fold can be issued before the last block lands.
DVE_BLOCKS = {0, 2, 4, 6, 8, 10, 12, 14}
# Alternate tile loads between the two HWDGE rings (SP and ACT queues).
SPLIT_DMA_QUEUES = True
# Additionally offload group 2 of PE blocks to the DVE (measured slower:
# the extra per-block DVE op + drain outweighs the PE savings).
STEAL_G2 = False

_cache = {}


def _window(blk):
    """Union of valid j-ranges for rows [r0, r0+127]: j must satisfy
    0 <= r + (j - H) <= T-1 for some row r in the block."""
    r0 = blk * P
    w0 = max(0, H - (r0 + P - 1))
    w1 = min(D, (H + T - 1) - r0 + 1)
    return w0, w1


def _build_nc():
    nc = bacc.Bacc(None, target_bir_lowering=False)
    x = nc.dram_tensor("x", [BPC, T, W], FP32, kind="ExternalInput")
    invc = nc.dram_tensor("invc", [1, D], FP32, kind="ExternalInput")
    out = nc.dram_tensor("out", [BPC, D], FP32, kind="ExternalOutput")

    groups = [(512 * g, min(512 * g + 512, D)) for g in range(5)]

    with tile.TileContext(nc) as tc:
        with (
            tc.tile_pool(name="consts", bufs=1) as consts,
            tc.tile_pool(name="tiles", bufs=12) as tiles,
            tc.tile_pool(name="psum", bufs=1, space="PSUM") as psum,
            tc.tile_pool(name="scratch", bufs=1, space="PSUM") as scratch_pool,
            tc.tile_pool(name="accp", bufs=2) as accp,
            tc.tile_pool(name="tail", bufs=2) as tail,
        ):
            ones = consts.tile([P, 1], FP32)
            nc.vector.memset(ones, 1.0)
            zeros = consts.tile([1, 1], FP32)
            nc.vector.memset(zeros, 0.0)
            invc_t = consts.tile([1, D], FP32)
            nc.sync.dma_start(out=invc_t, in_=invc[:, :])
            scratch = scratch_pool.tile([1, 1], FP32)

            # The fp32 self-loading matmul can carry at most ONE sync wait
            # before bacc has to split waits into event-semaphore chains.
            # Absorber matmuls pull cross-engine ticks into the PE vector
            # clock so real matmuls only ever wait on their tile's DMA.
            def absorb(dep_ap, out_ap=None, start=True):
                nc.tensor.matmul(
                    out=scratch[:, :] if out_ap is None else out_ap,
                    lhsT=ones[0:1, 0:1],
                    rhs=dep_ap,
                    start=start,
                    stop=True,
                    skip_group_check=True,
                )

            absorb(ones[0:1, 0:1])  # waits on the DVE memset

            res_tiles = []
            prev_means = None
            for b in range(BPC):
                ps = psum.tile([1, D], FP32)
                acc = accp.tile([P, D], FP32)
                nc.gpsimd.memset(acc, 0.0)
                if prev_means is not None:
                    # absorb the DVE read of the previous batch's PSUM so the
                    # next matmuls don't carry a WAR wait on DVE
                    absorb(prev_means[0:1, 0:1])
                    # absorb the PE-completion wait for reusing the PSUM banks
                    absorb(ones[0:1, 0:1], out_ap=ps[:, 0:1])
                # Zero every PSUM group with a full-width start=True matmul
                # against the freshly-memset accumulator. Trimmed block
                # matmuls can then accumulate at any sub-range: partial-width
                # start=True would leave a bank's pending-zero state mixed,
                # which is undefined on hardware.
                for c0, c1 in groups:
                    nc.tensor.matmul(
                        out=ps[:, c0:c1],
                        lhsT=ones[:, :],
                        rhs=acc[:, c0:c1],
                        start=True,
                        stop=False,
                        skip_group_check=True,
                    )
                def do_block(blk, steal_g2=STEAL_G2):
                    w0, w1 = _window(blk)
                    tl = tiles.tile([P, w1 - w0], FP32)
                    off = b * T * W + blk * P * (W + 1) + w0
                    src = bass.AP(
                        tensor=x, offset=off, ap=[[W + 1, P], [1, w1 - w0]]
                    )
                    eng = nc.scalar if (SPLIT_DMA_QUEUES and blk % 2) else nc.sync
                    eng.dma_start(out=tl[:, :], in_=src)
                    if blk in DVE_BLOCKS:
                        nc.vector.tensor_add(
                            out=acc[:, w0:w1], in0=acc[:, w0:w1], in1=tl[:, :]
                        )
                        return
                    # PE block: DVE still takes this block's group-2 slice to
                    # shave the PE (which would otherwise be the bottleneck)
                    # without ever leaving it idle long enough to re-throttle.
                    d0, d1 = max(1024, w0), min(1536, w1)
                    if steal_g2 and d0 < d1:
                        nc.vector.tensor_add(
                            out=acc[:, d0:d1],
                            in0=acc[:, d0:d1],
                            in1=tl[:, d0 - w0 : d1 - w0],
                        )
                    for g, (c0, c1) in enumerate(groups):
                        if g == 2 and steal_g2:
                            continue
                        i0, i1 = max(c0, w0), min(c1, w1)
                        if i0 >= i1:
                            continue
                        nc.tensor.matmul(
                            out=ps[:, i0:i1],
                            lhsT=ones[:, :],
                            rhs=tl[:, i0 - w0 : i1 - w0],
                            start=False,
                            stop=False,
                            skip_group_check=True,
                        )

                for blk in range(NBLK - 1):
                    do_block(blk)
                # fold the DVE accumulator into PSUM before the last block's
                # data has even landed (the acc is complete after block 14)
                for c0, c1 in groups:
                    nc.tensor.matmul(
                        out=ps[:, c0:c1],
                        lhsT=ones[:, :],
                        rhs=acc[:, c0:c1],
                        start=False,
                        stop=False,
                        skip_group_check=True,
                    )
                do_block(NBLK - 1, steal_g2=False)
                # close the accumulation (adds 0; stop is sim-side only)
                nc.tensor.matmul(
                    out=ps[:, 0:1],
                    lhsT=ones[0:1, 0:1],
                    rhs=zeros[:, :],
                    start=False,
                    stop=True,
                    skip_group_check=True,
                )
                means = tail.tile([1, D], FP32)
                ssum = tail.tile([1, 1], FP32)
                # one DVE pass: means_neg = ps * (-1/count), ssum = sum(means_neg)
                nc.vector.scalar_tensor_tensor(
                    out=means,
                    in0=ps[:, :],
                    scalar=1.0,
                    in1=invc_t,
                    op0=mybir.AluOpType.bypass,
                    op1=mybir.AluOpType.mult,
                    accum_out=ssum,
                )
                prev_means = means
                avg = tail.tile([1, 1], FP32)
                nc.scalar.mul(avg, ssum, 1.0 / D)
                res = tail.tile([1, D], FP32)
                nc.vector.tensor_scalar(
                    out=res,
                    in0=means,
                    scalar1=avg,
                    scalar2=None,
                    op0=mybir.AluOpType.subtract,
                )
                res_tiles.append(res)
            for b, res in enumerate(res_tiles):
                nc.sync.dma_start(out=out[b : b + 1, :], in_=res[:, :])
    nc.compile()
    return nc


def _prepare(x):
    """Pad rows to width W with the diagonal band centered, and zero the
    excluded (last) element of every diagonal."""
    x = np.ascontiguousarray(np.asarray(x, dtype=np.float32))
    assert x.shape == (B, T, T)
    xp = np.zeros((B, T, W), np.float32)
    xp[:, :, H : H + T] = x
    # d >= 0: excluded element is (T-1-d, T-1)
    rows = T - 1 - np.arange(0, H + 1)
    xp[:, rows, H + T - 1] = 0.0
    # d < 0: excluded element is (T-1, T-1+d)
    cols = T - 1 + np.arange(-H, 0)
    xp[:, T - 1, H + cols] = 0.0
    return xp


def _run(x, trace=False):
    if "nc" not in _cache:
        _cache["nc"] = _build_nc()
    nc = _cache["nc"]

    xp = _prepare(x)
    counts = (T - 1 - np.abs(np.arange(-H, H + 1))).astype(np.float32)
    invc = (-1.0 / counts).reshape(1, D)

    in_maps = [
        {"x": xp[c * BPC : (c + 1) * BPC], "invc": invc} for c in range(NCORES)
    ]
    r = run_bass_kernel_spmd(nc, in_maps, core_ids=list(range(NCORES)), trace=trace)
    out = np.concatenate([m["out"] for m in r.results], axis=0)
    return out, r.exec_time_ns


def kernel(inputs):
    out, _ = _run(inputs, trace=False)
    return out
